# revision 1
# baseline (speedup 1.0000x reference)
"""Entropic Sinkhorn loss kernel for Trainium2 (8 NeuronCores, SPMD).

Math (reference): loss = (sinkhorn(1 - img@txt.T) + sinkhorn((1 - img@txt.T).T)) / 2
with K = exp(-(M)/0.01) = exp(100*S - 100), S = img@txt.T, 5 Sinkhorn iterations,
then P = u * K * v, loss_half = -mean(log_softmax(P)[i, i]).

Device algorithm (per core, rows sharded B/8 = 1024):
  build: S row-shard GEMM (bf16) -> K = exp(100S - 100) (bf16) -> HBM, plus
         PE-transposed copy KT -> HBM; col-matvec of K with 1/n gives KtU_1.
  pass t (1..5), one pass over KT then K, all four matvecs on PE:
    row-matvecs (lhsT = KT blocks):  u_t = 1/(K v_t),  v'_t = b/(K u'_{t-1})
    col-matvecs (lhsT = K blocks):   KtU_{t+1} = K^T u_t -> v_{t+1} = b/AR(.)
                                     K^T v'_t -> u'_t = 1/AR(.)
    (AR = 64KB AllReduce across the 8 cores)
  loss: stream KT once; e1 = exp(v5_c * K[r,c] * u5_r) column-summed on PE
        (row-LSE of P1), e2 = exp(u'5_c * K[r,c] * v'5_r) row-summed via ACT
        accumulate (row-LSE of P2, needs AR); diagonal terms from host-provided
        K-diagonal; final scalar assembled identically on every core.
"""

import numpy as np
import ml_dtypes

import concourse.bass as bass
import concourse.bacc as bacc
import concourse.tile as tile
import concourse.mybir as mybir
from concourse import bass_utils
from concourse.bass import ds
from concourse.masks import make_identity

B = 8192
D = 256
P = 128
NCORES = 8
R = B // NCORES            # 1024 rows per core
RT = R // P                # 8 row tiles per core
CT = B // P                # 64 col tiles
NCH = B // 512             # 16 GEMM chunks of 512
N_ITER = 5
INV_REG = 100.0
BVAL = 1.0 / B

BF16 = mybir.dt.bfloat16
F32 = mybir.dt.float32
Exp = mybir.ActivationFunctionType.Exp
Log = mybir.ActivationFunctionType.Ln
ADD = mybir.AluOpType.add
MULT = mybir.AluOpType.mult


def _build_bass(b=None, phases=3, loss_stop=0, ncores=None):
    global B, R, RT, CT, NCH, BVAL, NCORES
    if ncores is not None:
        NCORES = ncores
    if b is not None:
        B = b
        R = B // NCORES
        RT = max(R // P, 1)
        CT = B // P
        NCH = max(B // 512, 1)
        BVAL = 1.0 / B
    nc = bacc.Bacc("TRN2", target_bir_lowering=False, debug=False,
                   num_devices=NCORES)

    imgT = nc.dram_tensor("imgT", [P, 2, R], BF16, kind="ExternalInput")
    txtT = nc.dram_tensor("txtT", [P, 2, B], BF16, kind="ExternalInput")
    kdiag_in = nc.dram_tensor("kdiag", [P, RT], F32, kind="ExternalInput")
    dsel_in = nc.dram_tensor("dsel", [P, RT, CT], F32, kind="ExternalInput")
    out_loss = nc.dram_tensor("out_loss", [1, 1], F32, kind="ExternalOutput")

    K_hbm = nc.dram_tensor("K_hbm", [R, B], BF16)
    KT_hbm = nc.dram_tensor("KT_hbm", [B, R], BF16)
    RG = [list(range(NCORES))]
    ar_in = [nc.dram_tensor(f"ar_in{t}", [P, 2 * CT], F32) for t in range(6)]
    ar_out = [nc.dram_tensor(f"ar_out{t}", [P, 2 * CT], F32, addr_space="Shared")
              for t in range(6)]
    ar2_in = nc.dram_tensor("ar2_in", [P, CT + 2], F32)
    ar2_out = nc.dram_tensor("ar2_out", [P, CT + 2], F32, addr_space="Shared")

    def allreduce(t, src_ap, dst_ap):
        if NCORES == 1:
            nc.gpsimd.dma_start(out=dst_ap, in_=src_ap)
            return
        nc.gpsimd.dma_start(out=ar_in[t][:], in_=src_ap)
        nc.gpsimd.collective_compute(
            "AllReduce", ADD, replica_groups=RG,
            ins=[ar_in[t][:]], outs=[ar_out[t][:]])
        nc.gpsimd.dma_start(out=dst_ap, in_=ar_out[t][:])

    with tile.TileContext(nc) as tc:
        with tc.tile_pool(name="persist", bufs=1) as pp:
            identity_bf = pp.tile([P, P], BF16, tag="identity")
            make_identity(nc, identity_bf[:])
            ones_bf = pp.tile([P, 1], BF16, tag="ones_bf")
            nc.vector.memset(ones_bf[:], 1.0)
            ones_f = pp.tile([P, 1], F32, tag="ones_f")
            nc.vector.memset(ones_f[:], 1.0)
            ones_row = pp.tile([1, P], BF16, tag="ones_row")
            nc.vector.memset(ones_row[:], 1.0)
            biasm100 = pp.tile([P, 1], F32, tag="biasm100")
            nc.vector.memset(biasm100[:], -INV_REG)
            kdiag_f = pp.tile([P, RT], F32, tag="kdiag")
            nc.sync.dma_start(out=kdiag_f[:], in_=kdiag_in[:])
            dsel_bf = pp.tile([P, RT, CT], F32, tag="dsel")
            nc.sync.dma_start(out=dsel_bf[:], in_=dsel_in[:])

            # per-pass persistent vectors
            # vt[t]: [p, 2*cb+vec] f32; vec0 = v_{t+1} (loss1), vec1 = u'_t (loss2)
            vt_f = [pp.tile([P, 2 * CT], F32, tag=f"vtf{t}", name=f"vtf{t}")
                    for t in range(6)]
            vt_bf = [pp.tile([P, 2 * CT], BF16, tag=f"vtbf{t}", name=f"vtbf{t}")
                     for t in range(6)]
            # uvrow (pass5): [p, 2*rb+vec] f32; vec0 = u_5, vec1 = v'_5
            uvrow5_f = pp.tile([P, 2 * RT], F32, tag="uvrow5f")
            uvrow5_bf = pp.tile([P, 2 * RT], BF16, tag="uvrow5bf")
            cse2_f = pp.tile([P, CT], F32, tag="cse2")

            # ---------------- build phase ----------------
            with tc.tile_pool(name="bfeat", bufs=1) as pf, \
                 tc.tile_pool(name="bk", bufs=2) as pk, \
                 tc.tile_pool(name="bkt", bufs=3) as pkt, \
                 tc.tile_pool(name="bps", bufs=2, space="PSUM") as pps, \
                 tc.tile_pool(name="bpt", bufs=2, space="PSUM") as ppt, \
                 tc.tile_pool(name="bpc", bufs=1, space="PSUM") as ppc:
                imgT_sb = pf.tile([P, 2, R], BF16, tag="imgT")
                txtT_sb = pf.tile([P, 2, B], BF16, tag="txtT")
                nc.sync.dma_start(out=imgT_sb[:], in_=imgT[:])
                nc.sync.dma_start(out=txtT_sb[:], in_=txtT[:])
                invn_bf = pf.tile([P, 2], BF16, tag="invn")
                nc.vector.memset(invn_bf[:], BVAL)

                psum_cs = ppc.tile([P, 2 * CT], F32, tag="psum_cs")
                for ri in range(RT):
                    ktile = pk.tile([P, B], BF16, tag="ktile")
                    for cj in range(NCH):
                        psum_s = pps.tile([P, 512], F32, tag="psum_s")
                        for dhi in range(2):
                            nc.tensor.matmul(
                                psum_s[:],
                                lhsT=imgT_sb[:, dhi, ri * P:(ri + 1) * P],
                                rhs=txtT_sb[:, dhi, cj * 512:(cj + 1) * 512],
                                start=(dhi == 0), stop=(dhi == 1))
                        nc.scalar.activation(
                            out=ktile[:, cj * 512:(cj + 1) * 512],
                            in_=psum_s[:], func=Exp,
                            scale=INV_REG, bias=biasm100[:])
                    nc.sync.dma_start(out=K_hbm[ri * P:(ri + 1) * P, :],
                                      in_=ktile[:])
                    # col-matvec with constant u0 = 1/n -> KtU_1 accumulation
                    for cb in range(CT):
                        nc.tensor.matmul(
                            psum_cs[:, 2 * cb:2 * cb + 2],
                            lhsT=ktile[:, cb * P:(cb + 1) * P],
                            rhs=invn_bf[:],
                            start=(ri == 0 and cb == 0),
                            stop=(ri == RT - 1 and cb == CT - 1),
                            skip_group_check=True)
                    # transposes: groups of 8 col-blocks
                    for g in range(CT // 8):
                        psum_t = ppt.tile([P, 1024], BF16, tag="psum_t")
                        for j in range(8):
                            cb = g * 8 + j
                            nc.tensor.transpose(
                                psum_t[:, j * P:(j + 1) * P],
                                ktile[:, cb * P:(cb + 1) * P],
                                identity_bf[:])
                        ktstage = pkt.tile([P, 1024], BF16, tag="ktstage")
                        if g % 2 == 0:
                            nc.scalar.copy(out=ktstage[:], in_=psum_t[:])
                        else:
                            nc.vector.tensor_copy(ktstage[:], psum_t[:])
                        dst = KT_hbm[g * 1024:(g + 1) * 1024,
                                     ri * P:(ri + 1) * P]
                        dst = dst.rearrange("(j p) r -> p j r", p=P)
                        src = ktstage[:].rearrange("p (j r) -> p j r", j=8)
                        nc.sync.dma_start(out=dst, in_=src)

                # build epilogue: AR(KtU_1) -> v_1 = b/KtU_1 ; u'_0 = 1/n
                cs_sb = pf.tile([P, 2 * CT], F32, tag="cs_sb")
                nc.scalar.copy(out=cs_sb[:], in_=psum_cs[:])
                g_sb = pf.tile([P, 2 * CT], F32, tag="g_sb")
                allreduce(0, cs_sb[:], g_sb[:])
                rec = pf.tile([P, 2 * CT], F32, tag="rec")
                nc.vector.reciprocal(rec[:], g_sb[:])
                v0 = vt_f[0][:].rearrange("p (c v) -> p c v", v=2)
                r0 = rec[:].rearrange("p (c v) -> p c v", v=2)
                nc.scalar.mul(out=v0[:, :, 0], in_=r0[:, :, 0], mul=BVAL)
                nc.vector.memset(v0[:, :, 1], BVAL)
                nc.vector.tensor_copy(vt_bf[0][:], vt_f[0][:])

            if phases < 2:
                dbg = pp.tile([P, 1], F32, tag="dbg")
                nc.vector.tensor_reduce(dbg[:], vt_f[0][:],
                                        axis=mybir.AxisListType.X, op=ADD)
                nc.sync.dma_start(out=out_loss[:], in_=dbg[0:1, 0:1])

            # ---------------- sinkhorn passes ----------------
            with tc.tile_pool(name="skt", bufs=4) as pskt, \
                 tc.tile_pool(name="sk", bufs=2) as psk, \
                 tc.tile_pool(name="ssm", bufs=2) as psm, \
                 tc.tile_pool(name="spr", bufs=2, space="PSUM") as ppr, \
                 tc.tile_pool(name="spc", bufs=2, space="PSUM") as ppcol:
                for t in range(1, (N_ITER + 1) if phases >= 2 else 1):
                    # stage A: row-matvecs over KT tiles
                    psum_r = ppr.tile([P, 2 * RT], F32, tag="psum_r")
                    for ct in range(CT):
                        kt_t = pskt.tile([P, R], BF16, tag="kt_t")
                        nc.sync.dma_start(
                            out=kt_t[:], in_=KT_hbm[ct * P:(ct + 1) * P, :])
                        for rb in range(RT):
                            nc.tensor.matmul(
                                psum_r[:, 2 * rb:2 * rb + 2],
                                lhsT=kt_t[:, rb * P:(rb + 1) * P],
                                rhs=vt_bf[t - 1][:, 2 * ct:2 * ct + 2],
                                start=(ct == 0 and rb == 0),
                                stop=(ct == CT - 1 and rb == RT - 1),
                                skip_group_check=True)
                    # u_t = 1/(K v_t); v'_t = b/(K u'_{t-1})
                    if t == N_ITER:
                        uv_f, uv_bf = uvrow5_f, uvrow5_bf
                    else:
                        uv_f = psm.tile([P, 2 * RT], F32, tag="uv_f")
                        uv_bf = psm.tile([P, 2 * RT], BF16, tag="uv_bf")
                    rr = psm.tile([P, 2 * RT], F32, tag="rr")
                    nc.vector.reciprocal(rr[:], psum_r[:])
                    rrv = rr[:].rearrange("p (r v) -> p r v", v=2)
                    uvv = uv_f[:].rearrange("p (r v) -> p r v", v=2)
                    nc.scalar.copy(out=uvv[:, :, 0], in_=rrv[:, :, 0])
                    nc.scalar.mul(out=uvv[:, :, 1], in_=rrv[:, :, 1], mul=BVAL)
                    nc.vector.tensor_copy(uv_bf[:], uv_f[:])

                    # stage B: col-matvecs over K tiles
                    psum_c = ppcol.tile([P, 2 * CT], F32, tag="psum_c")
                    for ri in range(RT):
                        k_t = psk.tile([P, B], BF16, tag="k_t")
                        nc.sync.dma_start(
                            out=k_t[:], in_=K_hbm[ri * P:(ri + 1) * P, :])
                        for cb in range(CT):
                            nc.tensor.matmul(
                                psum_c[:, 2 * cb:2 * cb + 2],
                                lhsT=k_t[:, cb * P:(cb + 1) * P],
                                rhs=uv_bf[:, 2 * ri:2 * ri + 2],
                                start=(ri == 0 and cb == 0),
                                stop=(ri == RT - 1 and cb == CT - 1),
                                skip_group_check=True)
                    # epilogue: AR -> v_{t+1} = b/KtU ; u'_t = 1/(K^T v'_t)
                    cs2 = psm.tile([P, 2 * CT], F32, tag="cs2")
                    nc.scalar.copy(out=cs2[:], in_=psum_c[:])
                    gg = psm.tile([P, 2 * CT], F32, tag="gg")
                    allreduce(t, cs2[:], gg[:])
                    rec2 = psm.tile([P, 2 * CT], F32, tag="rec2")
                    nc.vector.reciprocal(rec2[:], gg[:])
                    vv = vt_f[t][:].rearrange("p (c v) -> p c v", v=2)
                    rv = rec2[:].rearrange("p (c v) -> p c v", v=2)
                    nc.scalar.mul(out=vv[:, :, 0], in_=rv[:, :, 0], mul=BVAL)
                    nc.scalar.copy(out=vv[:, :, 1], in_=rv[:, :, 1])
                    nc.vector.tensor_copy(vt_bf[t][:], vt_f[t][:])

            if phases == 2:
                dbg = pp.tile([P, 1], F32, tag="dbg")
                nc.vector.tensor_reduce(dbg[:], uvrow5_f[:],
                                        axis=mybir.AxisListType.X, op=ADD)
                nc.sync.dma_start(out=out_loss[:], in_=dbg[0:1, 0:1])

            # ---------------- loss phase ----------------
            if phases >= 3:
              with tc.tile_pool(name="lkt", bufs=4) as plkt, \
                   tc.tile_pool(name="lm", bufs=3) as plm, \
                   tc.tile_pool(name="lsm", bufs=1) as pls, \
                   tc.tile_pool(name="lp1", bufs=1, space="PSUM") as ppl1, \
                   tc.tile_pool(name="lpf", bufs=1, space="PSUM") as pplf, \
                   tc.tile_pool(name="lpb", bufs=1, space="PSUM") as pplb:
                  # replicate u5 / v'5 along partitions: [p, r] = x[r]
                  reps = []
                  for vec in range(2):
                      psum_fl = pplf.tile([1, R], F32, tag="psum_fl")
                      for rb in range(RT):
                          nc.tensor.matmul(
                              psum_fl[0:1, rb * P:(rb + 1) * P],
                              lhsT=uvrow5_bf[:, 2 * rb + vec:2 * rb + vec + 1],
                              rhs=identity_bf[:],
                              start=(rb % 4 == 0),
                              stop=(rb % 4 == 3 or rb == RT - 1),
                              skip_group_check=True)
                      flat_bf = pls.tile([1, R], BF16, tag=f"flat{vec}")
                      nc.scalar.copy(out=flat_bf[:], in_=psum_fl[:])
                      rep = pls.tile([P, R], BF16, tag=f"rep{vec}",
                                     name=f"rep{vec}")
                      bcch = min(512, R)
                      for h in range(R // bcch):
                          sl = slice(h * bcch, (h + 1) * bcch)
                          psum_bc = pplb.tile([P, bcch], F32, tag="psum_bc",
                                              bufs=2 if R <= 1024 else 1,
                                              name="psum_bc")
                          nc.tensor.matmul(
                              psum_bc[:],
                              lhsT=ones_row[:],
                              rhs=flat_bf[0:1, sl],
                              start=True, stop=True)
                          if h % 2 == 0:
                              nc.scalar.copy(out=rep[:, sl], in_=psum_bc[:])
                          else:
                              nc.vector.tensor_copy(rep[:, sl], psum_bc[:])
                      reps.append(rep)
                  u5_rep, vp5_rep = reps

                  # diagonal terms (host kdiag + mask-select of v5 / u'5 columns)
                  if loss_stop in (0, 2, 3):
                      v4v = vt_f[4][:].rearrange("p (c v) -> p c v", v=2)
                      v5v = vt_f[5][:].rearrange("p (c v) -> p c v", v=2)
                      uv5 = uvrow5_f[:].rearrange("p (r v) -> p r v", v=2)
                      v5d = pls.tile([P, RT], F32, tag="v5d")
                      up5d = pls.tile([P, RT], F32, tag="up5d")
                      selscr = pls.tile([P, CT], F32, tag="selscr")
                      for rb in range(RT):
                          nc.vector.tensor_mul(selscr[:], v4v[:, :, 0],
                                               dsel_bf[:, rb, :])
                          nc.vector.tensor_reduce(
                              v5d[:, rb:rb + 1], selscr[:],
                              axis=mybir.AxisListType.X, op=ADD)
                          nc.vector.tensor_mul(selscr[:], v5v[:, :, 1],
                                               dsel_bf[:, rb, :])
                          nc.vector.tensor_reduce(
                              up5d[:, rb:rb + 1], selscr[:],
                              axis=mybir.AxisListType.X, op=ADD)
                      pd1 = pls.tile([P, RT], F32, tag="pd1")
                      pd2 = pls.tile([P, RT], F32, tag="pd2")
                      nc.vector.tensor_mul(pd1[:], uv5[:, :, 0], kdiag_f[:])
                      nc.vector.tensor_mul(pd1[:], pd1[:], v5d[:])
                      nc.vector.tensor_mul(pd2[:], uv5[:, :, 1], kdiag_f[:])
                      nc.vector.tensor_mul(pd2[:], pd2[:], up5d[:])

                  if loss_stop == 2:
                      dbg = pp.tile([P, 1], F32, tag="dbg")
                      nc.vector.tensor_reduce(dbg[:], pd1[:], axis=mybir.AxisListType.X, op=ADD)
                      nc.sync.dma_start(out=out_loss[:], in_=dbg[0:1, 0:1])
                  if loss_stop in (0, 3):
                      # main streamed loop over KT
                      psum_l1 = ppl1.tile([P, RT], F32, tag="psum_l1")
                      for ct in range(CT):
                          kt_t = plkt.tile([P, R], BF16, tag="lkt_t")
                          nc.sync.dma_start(
                              out=kt_t[:], in_=KT_hbm[ct * P:(ct + 1) * P, :])
                          m1 = plm.tile([P, R], BF16, tag="m1")
                          nc.vector.tensor_mul(m1[:], kt_t[:], u5_rep[:])
                          e1 = plm.tile([P, R], BF16, tag="e1")
                          nc.scalar.activation(out=e1[:], in_=m1[:], func=Exp,
                                               scale=v4v[:, ct, 0:1])
                          for rb in range(RT):
                              nc.tensor.matmul(
                                  psum_l1[:, rb:rb + 1],
                                  lhsT=e1[:, rb * P:(rb + 1) * P],
                                  rhs=ones_bf[:],
                                  start=(ct == 0 and rb == 0),
                                  stop=(ct == CT - 1 and rb == RT - 1),
                                  skip_group_check=True)
                          m2 = plm.tile([P, R], BF16, tag="m2")
                          nc.vector.tensor_mul(m2[:], kt_t[:], vp5_rep[:])
                          e2 = plm.tile([P, R], BF16, tag="e2")
                          nc.scalar.activation(out=e2[:], in_=m2[:], func=Exp,
                                               scale=v5v[:, ct, 1:2],
                                               accum_out=cse2_f[:, ct:ct + 1])

                  if loss_stop == 3:
                      dbg = pp.tile([P, 1], F32, tag="dbg")
                      nc.vector.tensor_reduce(dbg[:], cse2_f[:], axis=mybir.AxisListType.X, op=ADD)
                      nc.sync.dma_start(out=out_loss[:], in_=dbg[0:1, 0:1])
                  if loss_stop == 0:
                      # loss1 local total: sum_p sum_rb (log(sum e1) - pd1)
                      lse1 = pls.tile([P, RT], F32, tag="lse1")
                      nc.scalar.activation(out=lse1[:], in_=psum_l1[:], func=Log)
                      d1 = pls.tile([P, RT], F32, tag="d1")
                      nc.vector.tensor_sub(d1[:], lse1[:], pd1[:])
                      pack2 = pls.tile([P, 2], F32, tag="pack2")
                      nc.vector.tensor_reduce(pack2[:, 0:1], d1[:],
                                              axis=mybir.AxisListType.X, op=ADD)
                      nc.vector.tensor_reduce(pack2[:, 1:2], pd2[:],
                                              axis=mybir.AxisListType.X, op=ADD)
                      psum_sc = pplb.tile([1, 2], F32, tag="psum_sc")
                      nc.tensor.matmul(psum_sc[:], lhsT=ones_f[:], rhs=pack2[:],
                                       start=True, stop=True)

                      # second AR: cse2 partials + the two scalars
                      stage2 = pls.tile([P, CT + 2], F32, tag="stage2")
                      nc.vector.memset(stage2[:], 0.0)
                      nc.vector.tensor_copy(stage2[:, 0:CT], cse2_f[:])
                      nc.scalar.copy(out=stage2[0:1, CT:CT + 2], in_=psum_sc[:])
                      g2 = pls.tile([P, CT + 2], F32, tag="g2")
                      if NCORES == 1:
                          nc.gpsimd.dma_start(out=g2[:], in_=stage2[:])
                      else:
                          nc.gpsimd.dma_start(out=ar2_in[:], in_=stage2[:])
                          nc.gpsimd.collective_compute(
                              "AllReduce", ADD, replica_groups=RG,
                              ins=[ar2_in[:]], outs=[ar2_out[:]])
                          nc.gpsimd.dma_start(out=g2[:], in_=ar2_out[:])

                      lse2 = pls.tile([P, CT], F32, tag="lse2")
                      nc.scalar.activation(out=lse2[:], in_=g2[:, 0:CT], func=Log)
                      l2s = pls.tile([P, 1], F32, tag="l2s")
                      nc.vector.tensor_reduce(l2s[:], lse2[:],
                                              axis=mybir.AxisListType.X, op=ADD)
                      psum_fs = pplb.tile([1, 1], F32, tag="psum_fs")
                      nc.tensor.matmul(psum_fs[:], lhsT=ones_f[:], rhs=l2s[:],
                                       start=True, stop=True)
                      fin = pls.tile([1, 1], F32, tag="fin")
                      nc.scalar.copy(out=fin[:], in_=psum_fs[:])
                      nc.vector.tensor_add(fin[:], fin[:], g2[0:1, CT:CT + 1])
                      nc.vector.tensor_sub(fin[:], fin[:], g2[0:1, CT + 1:CT + 2])
                      nc.scalar.mul(out=fin[:], in_=fin[:], mul=1.0 / (2 * B))
                      nc.sync.dma_start(out=out_loss[:], in_=fin[:])

    nc.compile()
    return nc


_NC_CACHE = None


def _get_nc():
    global _NC_CACHE
    if _NC_CACHE is None:
        _NC_CACHE = _build_bass()
    return _NC_CACHE


def make_in_maps(all_image_features, all_text_features):
    img = np.asarray(all_image_features, np.float32)
    txt = np.asarray(all_text_features, np.float32)

    img_bf = img.astype(ml_dtypes.bfloat16)
    txt_bf = txt.astype(ml_dtypes.bfloat16)
    # [d, x] -> [dlo, dhi, x] with d = dhi*128 + dlo
    imgT = np.ascontiguousarray(
        img_bf.T.reshape(2, P, B).transpose(1, 0, 2))
    txtT = np.ascontiguousarray(
        txt_bf.T.reshape(2, P, B).transpose(1, 0, 2))

    # host-side K diagonal (consistent with bf16 GEMM inputs, fp32 exp)
    sdiag = np.einsum("bd,bd->b",
                      img_bf.astype(np.float32), txt_bf.astype(np.float32))
    kdiag = np.exp(INV_REG * sdiag - INV_REG).astype(np.float32)

    in_maps = []
    for c in range(NCORES):
        rows = slice(c * R, (c + 1) * R)
        kd = np.ascontiguousarray(
            kdiag[rows].reshape(RT, P).T).astype(np.float32)  # [p, rb]
        # dsel[p, rb, cb] = 1 iff cb == c*RT + rb  (same for all p)
        dsel = np.zeros((P, RT, CT), np.float32)
        for rb in range(RT):
            dsel[:, rb, c * RT + rb] = 1.0
        in_maps.append({
            "imgT": np.ascontiguousarray(imgT[:, :, rows]),
            "txtT": txtT,
            "kdiag": kd,
            "dsel": dsel,
        })
    return in_maps


def kernel(all_image_features, all_text_features, logit_scale, labels):
    in_maps = make_in_maps(all_image_features, all_text_features)
    nc = _get_nc()
    res = bass_utils.run_bass_kernel_spmd(
        nc, in_maps, core_ids=list(range(NCORES)))
    loss = res.results[0]["out_loss"][0, 0]
    return np.asarray(loss, dtype=np.float32)



# revision 5
# speedup vs baseline: 1.7883x; 1.7883x over previous
"""Entropic Sinkhorn loss kernel for Trainium2 (8 NeuronCores, SPMD).

Math (reference): loss = (sinkhorn(1 - img@txt.T) + sinkhorn((1 - img@txt.T).T)) / 2
with K = exp(-(M)/0.01) = exp(100*S - 100), S = img@txt.T, 5 Sinkhorn iterations,
then P = u * K * v, loss_half = -mean(log_softmax(P)[i, i]).

v2 design (per core, rows sharded B/8 = 1024):
  * K^T kept SBUF-resident in bf16 ([128 part = c%128, 64 cb, 1024 r] =
    128KB/partition). No K/KT HBM streaming at all.
  * build: S^T tiles via GEMM (lhsT=txtT, rhs=imgT) -> ACT exp(100S-100) with
    accum_out accumulating row-sums -> KtU_1 = K^T u0 partials for free.
  * pass t (1..5):
      stage A (PE): row-matvecs u_t = 1/(K v_t), v'_t = b/(K u'_{t-1}) as
        512 [128x128]@[128x2] matmuls against resident K^T tiles.
      replicate u_t, v'_t across partitions (PE flatten + broadcast).
      stage B (DVE): col-matvecs K^T u_t / K^T v'_t as fused multiply+reduce
        (scalar_tensor_tensor, 4x DVE mode) -> 64KB AllReduce -> v_{t+1}, u'_t.
  * loss: P1 = u5*K*v5 rows sum to exactly 1 (u = 1/(K v) by construction), and
    all P entries are in [0,1], so LSE_i = log(n + exp(P_ii) - P_ii) up to a
    provable error < 0.72/n absolute. Only diagonal terms are needed: no pass
    over K. Diagonals come from host kdiag + dsel mask-select of v5/u'5.
"""

import numpy as np
import ml_dtypes

import concourse.bass as bass
import concourse.bacc as bacc
import concourse.tile as tile
import concourse.mybir as mybir
from concourse import bass_utils
from concourse.masks import make_identity

B = 8192
D = 256
P = 128
NCORES = 8
R = B // NCORES            # 1024 rows per core
RT = R // P                # 8 row tiles per core
CT = B // P                # 64 col tiles
N_ITER = 5
INV_REG = 100.0
BVAL = 1.0 / B

BF16 = mybir.dt.bfloat16
F32 = mybir.dt.float32
Exp = mybir.ActivationFunctionType.Exp
Log = mybir.ActivationFunctionType.Ln
ADD = mybir.AluOpType.add
MULT = mybir.AluOpType.mult


def _build_bass(b=None, phases=3, ncores=None):
    global B, R, RT, CT, BVAL, NCORES
    if ncores is not None:
        NCORES = ncores
    if b is not None:
        B = b
        R = B // NCORES
        RT = max(R // P, 1)
        CT = B // P
        BVAL = 1.0 / B
    RCH = min(R, 512)          # build GEMM free chunk
    NRC = R // RCH             # chunks per c-tile (2 for full size)
    nc = bacc.Bacc("TRN2", target_bir_lowering=False, debug=False,
                   num_devices=NCORES)

    imgT = nc.dram_tensor("imgT", [P, 2, R], BF16, kind="ExternalInput")
    txtT = nc.dram_tensor("txtT", [P, 2, B], BF16, kind="ExternalInput")
    kdiag_in = nc.dram_tensor("kdiag", [P, RT], F32, kind="ExternalInput")
    dsel_in = nc.dram_tensor("dsel", [P, RT, CT], F32, kind="ExternalInput")
    out_loss = nc.dram_tensor("out_loss", [1, 1], F32, kind="ExternalOutput")

    RG = [list(range(NCORES))]
    # AR staging: t=0 and t=5 carry one chain ([P, CT]); t=1..4 carry both.
    ar_sz = [CT if t in (0, N_ITER) else 2 * CT for t in range(N_ITER + 1)]
    ar_in = [nc.dram_tensor(f"ar_in{t}", [P, ar_sz[t]], F32)
             for t in range(N_ITER + 1)]
    ar_out = [nc.dram_tensor(f"ar_out{t}", [P, ar_sz[t]], F32,
                             addr_space="Shared") for t in range(N_ITER + 1)]
    ar2_in = nc.dram_tensor("ar2_in", [P, 2], F32)
    ar2_out = nc.dram_tensor("ar2_out", [P, 2], F32, addr_space="Shared")

    def allreduce(t, src_ap, dst_ap):
        if NCORES == 1:
            nc.gpsimd.dma_start(out=dst_ap, in_=src_ap)
            return
        nc.gpsimd.dma_start(out=ar_in[t][:], in_=src_ap)
        nc.gpsimd.collective_compute(
            "AllReduce", ADD, replica_groups=RG,
            ins=[ar_in[t][:]], outs=[ar_out[t][:]])
        nc.gpsimd.dma_start(out=dst_ap, in_=ar_out[t][:])

    with tile.TileContext(nc) as tc:
        with tc.tile_pool(name="persist", bufs=1) as pp, \
             tc.tile_pool(name="kres", bufs=1) as pkr:
            identity_bf = pp.tile([P, P], BF16, tag="identity")
            make_identity(nc, identity_bf[:])
            ones_f = pp.tile([P, 1], F32, tag="ones_f")
            nc.vector.memset(ones_f[:], 1.0)
            ones_row = pp.tile([1, P], BF16, tag="ones_row")
            nc.vector.memset(ones_row[:], 1.0)
            biasm100 = pp.tile([P, 1], F32, tag="biasm100")
            nc.vector.memset(biasm100[:], -INV_REG)
            nbias = pp.tile([P, 1], F32, tag="nbias")
            nc.vector.memset(nbias[:], float(B))
            kdiag_f = pp.tile([P, RT], F32, tag="kdiag")
            nc.sync.dma_start(out=kdiag_f[:], in_=kdiag_in[:])
            dsel_f = pp.tile([P, RT, CT], F32, tag="dsel")
            nc.sync.dma_start(out=dsel_f[:], in_=dsel_in[:])

            # per-pass persistent vectors
            # vt[t]: [p, cb, 2] f32; vec0 = v_{t+1} (chain1), vec1 = u'_t (chain2)
            vt_f = [pp.tile([P, 2 * CT], F32, tag=f"vtf{t}", name=f"vtf{t}")
                    for t in range(N_ITER + 1)]
            vt_bf = [pp.tile([P, 2 * CT], BF16, tag=f"vtbf{t}", name=f"vtbf{t}")
                     for t in range(N_ITER)]
            # uvrow5: [p, rb, 2] f32; vec0 = u_5 (chain1), vec1 = v'_5 (chain2)
            uvrow5_f = pp.tile([P, 2 * RT], F32, tag="uvrow5f")

            # SBUF-resident K^T: [p, cb, r] with c = cb*128 + p
            kt_res = pkr.tile([P, CT, R], BF16, tag="ktres")

            # ---------------- build phase ----------------
            with tc.tile_pool(name="bfeat", bufs=1) as pf, \
                 tc.tile_pool(name="bps", bufs=4, space="PSUM") as pps:
                imgT_sb = pf.tile([P, 2, R], BF16, tag="imgT")
                txtT_sb = pf.tile([P, 2, B], BF16, tag="txtT")
                nc.sync.dma_start(out=imgT_sb[:], in_=imgT[:])
                # split the 4MB txtT load across DMA queues
                ntch = min(8, max(1, B // 1024))
                tch = B // ntch
                for k in range(ntch):
                    nc.sync.dma_start(
                        out=txtT_sb[:, :, k * tch:(k + 1) * tch],
                        in_=txtT[:, :, k * tch:(k + 1) * tch])

                kslots = pf.tile([P, CT * NRC], F32, tag="kslots")
                for cb in range(CT):
                    for h in range(NRC):
                        psum_s = pps.tile([P, RCH], F32, tag="psum_s")
                        for dh in range(2):
                            nc.tensor.matmul(
                                psum_s[:],
                                lhsT=txtT_sb[:, dh, cb * P:(cb + 1) * P],
                                rhs=imgT_sb[:, dh, h * RCH:(h + 1) * RCH],
                                start=(dh == 0), stop=(dh == 1))
                        nc.scalar.activation(
                            out=kt_res[:, cb, h * RCH:(h + 1) * RCH],
                            in_=psum_s[:], func=Exp,
                            scale=INV_REG, bias=biasm100[:],
                            accum_out=kslots[:, cb * NRC + h:cb * NRC + h + 1])

                # KtU_1 partials = (sum of row-sum slots) * (1/n)
                ktu1 = pf.tile([P, CT], F32, tag="ktu1")
                if NRC == 2:
                    ks3 = kslots[:].rearrange("p (c h) -> p c h", h=2)
                    nc.vector.tensor_add(ktu1[:], ks3[:, :, 0], ks3[:, :, 1])
                    nc.scalar.mul(out=ktu1[:], in_=ktu1[:], mul=BVAL)
                else:
                    nc.scalar.mul(out=ktu1[:], in_=kslots[:], mul=BVAL)
                g0 = pf.tile([P, CT], F32, tag="g0")
                allreduce(0, ktu1[:], g0[:])
                rec0 = pf.tile([P, CT], F32, tag="rec0")
                nc.vector.reciprocal(rec0[:], g0[:])
                v0 = vt_f[0][:].rearrange("p (c v) -> p c v", v=2)
                nc.scalar.mul(out=v0[:, :, 0], in_=rec0[:], mul=BVAL)
                nc.vector.memset(v0[:, :, 1], BVAL)
                nc.vector.tensor_copy(vt_bf[0][:], vt_f[0][:])

            if phases < 2:
                dbg = pp.tile([P, 1], F32, tag="dbg")
                nc.vector.tensor_reduce(dbg[:], vt_f[0][:],
                                        axis=mybir.AxisListType.X, op=ADD)
                nc.sync.dma_start(out=out_loss[:], in_=dbg[0:1, 0:1])

            # ---------------- sinkhorn passes ----------------
            with tc.tile_pool(name="srep", bufs=2) as prep, \
                 tc.tile_pool(name="sscr", bufs=2) as pscr, \
                 tc.tile_pool(name="ssm", bufs=2) as psm, \
                 tc.tile_pool(name="spr", bufs=2, space="PSUM") as ppr, \
                 tc.tile_pool(name="spf", bufs=2, space="PSUM") as pfl, \
                 tc.tile_pool(name="spb", bufs=2, space="PSUM") as pbc:
                for t in range(1, (N_ITER + 1) if phases >= 2 else 1):
                    # ---- stage A: row matvecs on PE ----
                    psum_r = ppr.tile([P, 2 * RT], F32, tag="psum_r")
                    for cb in range(CT):
                        for rb in range(RT):
                            nc.tensor.matmul(
                                psum_r[:, 2 * rb:2 * rb + 2],
                                lhsT=kt_res[:, cb, rb * P:(rb + 1) * P],
                                rhs=vt_bf[t - 1][:, 2 * cb:2 * cb + 2],
                                start=(cb == 0 and rb == 0),
                                stop=(cb == CT - 1 and rb == RT - 1),
                                skip_group_check=True)
                    # u_t = 1/(K v_t); v'_t = b/(K u'_{t-1})
                    if t == N_ITER:
                        uv_f = uvrow5_f
                    else:
                        uv_f = psm.tile([P, 2 * RT], F32, tag="uv_f")
                    uv_bf = psm.tile([P, 2 * RT], BF16, tag="uv_bf")
                    rr = psm.tile([P, 2 * RT], F32, tag="rr")
                    nc.vector.reciprocal(rr[:], psum_r[:])
                    rrv = rr[:].rearrange("p (r v) -> p r v", v=2)
                    uvv = uv_f[:].rearrange("p (r v) -> p r v", v=2)
                    nc.scalar.copy(out=uvv[:, :, 0], in_=rrv[:, :, 0])
                    nc.scalar.mul(out=uvv[:, :, 1], in_=rrv[:, :, 1], mul=BVAL)
                    nc.vector.tensor_copy(uv_bf[:], uv_f[:])

                    # ---- replicate u_t / v'_t across partitions ----
                    reps = []
                    for vec in range(2):
                        psum_fl = pfl.tile([1, R], F32, tag="psum_fl")
                        for rb in range(RT):
                            nc.tensor.matmul(
                                psum_fl[0:1, rb * P:(rb + 1) * P],
                                lhsT=uv_bf[:, 2 * rb + vec:2 * rb + vec + 1],
                                rhs=identity_bf[:],
                                start=(rb % 4 == 0),
                                stop=(rb % 4 == 3 or rb == RT - 1),
                                skip_group_check=True)
                        flat_bf = psm.tile([1, R], BF16, tag=f"flat{vec}")
                        nc.scalar.copy(out=flat_bf[:], in_=psum_fl[:])
                        rep = prep.tile([P, R], BF16, tag=f"rep{vec}",
                                        name=f"rep{vec}")
                        bcch = min(512, R)
                        for h in range(R // bcch):
                            sl = slice(h * bcch, (h + 1) * bcch)
                            psum_bc = pbc.tile([P, bcch], F32, tag="psum_bc")
                            nc.tensor.matmul(
                                psum_bc[:],
                                lhsT=ones_row[:],
                                rhs=flat_bf[0:1, sl],
                                start=True, stop=True)
                            if h % 2 == 0:
                                nc.scalar.copy(out=rep[:, sl], in_=psum_bc[:])
                            else:
                                nc.vector.tensor_copy(rep[:, sl], psum_bc[:])
                        reps.append(rep)
                    u_rep, vp_rep = reps

                    # ---- stage B: col matvecs as fused mul+reduce on DVE ----
                    nch = 1 if t == N_ITER else 2      # chains this pass
                    ktu_acc = psm.tile([P, nch * CT], F32, tag=f"ktu{nch}",
                                       name=f"ktu{nch}")
                    for cb in range(CT):
                        if t < N_ITER:
                            scr = pscr.tile([P, R], BF16, tag="scr")
                            nc.vector.scalar_tensor_tensor(
                                out=scr[:], in0=kt_res[:, cb, :], scalar=1.0,
                                in1=u_rep[:], op0=MULT, op1=MULT,
                                accum_out=ktu_acc[:, nch * cb:nch * cb + 1])
                        scr2 = pscr.tile([P, R], BF16, tag="scr2")
                        nc.vector.scalar_tensor_tensor(
                            out=scr2[:], in0=kt_res[:, cb, :], scalar=1.0,
                            in1=vp_rep[:], op0=MULT, op1=MULT,
                            accum_out=ktu_acc[:, nch * cb + nch - 1:
                                              nch * cb + nch])

                    gg = psm.tile([P, nch * CT], F32, tag=f"gg{nch}",
                                  name=f"gg{nch}")
                    allreduce(t, ktu_acc[:], gg[:])
                    rec2 = psm.tile([P, nch * CT], F32, tag=f"rec2{nch}",
                                    name=f"rec2{nch}")
                    nc.vector.reciprocal(rec2[:], gg[:])
                    vv = vt_f[t][:].rearrange("p (c v) -> p c v", v=2)
                    if t < N_ITER:
                        rv = rec2[:].rearrange("p (c v) -> p c v", v=2)
                        nc.scalar.mul(out=vv[:, :, 0], in_=rv[:, :, 0],
                                      mul=BVAL)
                        nc.scalar.copy(out=vv[:, :, 1], in_=rv[:, :, 1])
                        nc.vector.tensor_copy(vt_bf[t][:], vt_f[t][:])
                    else:
                        # only chain2 (u'_5) is needed downstream
                        nc.scalar.copy(out=vv[:, :, 1], in_=rec2[:])

            # ---------------- loss phase (diagonal only) ----------------
            if phases >= 3:
              with tc.tile_pool(name="lsm", bufs=1) as pls, \
                   tc.tile_pool(name="lpb", bufs=1, space="PSUM") as pplb:
                v4v = vt_f[N_ITER - 1][:].rearrange("p (c v) -> p c v", v=2)
                v5v = vt_f[N_ITER][:].rearrange("p (c v) -> p c v", v=2)
                uv5 = uvrow5_f[:].rearrange("p (r v) -> p r v", v=2)
                # mask-select diagonal entries of v5 (chain1) / u'5 (chain2)
                v5d = pls.tile([P, RT], F32, tag="v5d")
                up5d = pls.tile([P, RT], F32, tag="up5d")
                selscr = pls.tile([P, CT], F32, tag="selscr")
                for rb in range(RT):
                    nc.vector.tensor_mul(selscr[:], v4v[:, :, 0],
                                         dsel_f[:, rb, :])
                    nc.vector.tensor_reduce(
                        v5d[:, rb:rb + 1], selscr[:],
                        axis=mybir.AxisListType.X, op=ADD)
                    nc.vector.tensor_mul(selscr[:], v5v[:, :, 1],
                                         dsel_f[:, rb, :])
                    nc.vector.tensor_reduce(
                        up5d[:, rb:rb + 1], selscr[:],
                        axis=mybir.AxisListType.X, op=ADD)
                # pd = u * K_ii * v  (diagonal of P), per chain
                pd1 = pls.tile([P, RT], F32, tag="pd1")
                pd2 = pls.tile([P, RT], F32, tag="pd2")
                nc.vector.tensor_mul(pd1[:], uv5[:, :, 0], kdiag_f[:])
                nc.vector.tensor_mul(pd1[:], pd1[:], v5d[:])
                nc.vector.tensor_mul(pd2[:], uv5[:, :, 1], kdiag_f[:])
                nc.vector.tensor_mul(pd2[:], pd2[:], up5d[:])

                # per-row loss term: log(n + exp(pd) - pd) - pd
                pack2 = pls.tile([P, 2], F32, tag="pack2")
                for ci, pd in enumerate((pd1, pd2)):
                    e = pls.tile([P, RT], F32, tag=f"e{ci}", name=f"e{ci}")
                    e2 = pls.tile([P, RT], F32, tag=f"e2{ci}", name=f"e2{ci}")
                    nc.scalar.activation(out=e[:], in_=pd[:], func=Exp)
                    nc.vector.tensor_sub(e[:], e[:], pd[:])
                    # log(n + (exp(pd) - pd)) via Ln bias
                    nc.scalar.activation(out=e2[:], in_=e[:], func=Log,
                                         bias=nbias[:])
                    nc.vector.tensor_sub(e2[:], e2[:], pd[:])
                    nc.vector.tensor_reduce(pack2[:, ci:ci + 1], e2[:],
                                            axis=mybir.AxisListType.X, op=ADD)
                psum_sc = pplb.tile([1, 2], F32, tag="psum_sc")
                nc.tensor.matmul(psum_sc[:], lhsT=ones_f[:], rhs=pack2[:],
                                 start=True, stop=True)
                stage2 = pls.tile([P, 2], F32, tag="stage2")
                nc.vector.memset(stage2[:], 0.0)
                nc.scalar.copy(out=stage2[0:1, 0:2], in_=psum_sc[:])
                g2 = pls.tile([P, 2], F32, tag="g2")
                if NCORES == 1:
                    nc.gpsimd.dma_start(out=g2[:], in_=stage2[:])
                else:
                    nc.gpsimd.dma_start(out=ar2_in[:], in_=stage2[:])
                    nc.gpsimd.collective_compute(
                        "AllReduce", ADD, replica_groups=RG,
                        ins=[ar2_in[:]], outs=[ar2_out[:]])
                    nc.gpsimd.dma_start(out=g2[:], in_=ar2_out[:])
                fin = pls.tile([1, 1], F32, tag="fin")
                nc.vector.tensor_add(fin[:], g2[0:1, 0:1], g2[0:1, 1:2])
                nc.scalar.mul(out=fin[:], in_=fin[:], mul=1.0 / (2 * B))
                nc.sync.dma_start(out=out_loss[:], in_=fin[:])
            elif phases == 2:
                dbg = pp.tile([P, 1], F32, tag="dbg")
                nc.vector.tensor_reduce(dbg[:], uvrow5_f[:],
                                        axis=mybir.AxisListType.X, op=ADD)
                nc.sync.dma_start(out=out_loss[:], in_=dbg[0:1, 0:1])

    nc.compile()
    return nc


_NC_CACHE = None


def _get_nc():
    global _NC_CACHE
    if _NC_CACHE is None:
        _NC_CACHE = _build_bass()
    return _NC_CACHE


def make_in_maps(all_image_features, all_text_features):
    img = np.asarray(all_image_features, np.float32)
    txt = np.asarray(all_text_features, np.float32)

    img_bf = img.astype(ml_dtypes.bfloat16)
    txt_bf = txt.astype(ml_dtypes.bfloat16)
    # [d, x] -> [dlo, dhi, x] with d = dhi*128 + dlo
    imgT = np.ascontiguousarray(
        img_bf.T.reshape(2, P, B).transpose(1, 0, 2))
    txtT = np.ascontiguousarray(
        txt_bf.T.reshape(2, P, B).transpose(1, 0, 2))

    # host-side K diagonal (consistent with bf16 GEMM inputs, fp32 exp)
    sdiag = np.einsum("bd,bd->b",
                      img_bf.astype(np.float32), txt_bf.astype(np.float32))
    kdiag = np.exp(INV_REG * sdiag - INV_REG).astype(np.float32)

    in_maps = []
    for c in range(NCORES):
        rows = slice(c * R, (c + 1) * R)
        kd = np.ascontiguousarray(
            kdiag[rows].reshape(RT, P).T).astype(np.float32)  # [p, rb]
        # dsel[p, rb, cb] = 1 iff cb == c*RT + rb  (same for all p)
        dsel = np.zeros((P, RT, CT), np.float32)
        for rb in range(RT):
            dsel[:, rb, c * RT + rb] = 1.0
        in_maps.append({
            "imgT": np.ascontiguousarray(imgT[:, :, rows]),
            "txtT": txtT,
            "kdiag": kd,
            "dsel": dsel,
        })
    return in_maps


def kernel(all_image_features, all_text_features, logit_scale, labels):
    in_maps = make_in_maps(all_image_features, all_text_features)
    nc = _get_nc()
    res = bass_utils.run_bass_kernel_spmd(
        nc, in_maps, core_ids=list(range(NCORES)))
    loss = res.results[0]["out_loss"][0, 0]
    return np.asarray(loss, dtype=np.float32)
